# revision 19
# baseline (speedup 1.0000x reference)
"""CDR-aligned conditioner kernel for Trainium2 (8 NeuronCores).

Strategy
--------
The reference projects every text token through a 2-layer MLP
(3584 -> 768 -> SiLU -> 384) and then, per (chain_type, cdr_type) pair,
copies the k-th masked text row to the k-th masked protein position.
Only protein positions that receive a row are nonzero in the output
(~2460 of 16384 rows), so:

1. (host) compute the aligned (batch, text_src, protein_dst) triples
   with cheap integer ops — exactly the reference's cumsum/rank logic;
2. (host) gather just those text rows;
3. (device, 8 cores data-parallel over rows) dense MLP on the gathered
   rows:  Y^T = (scale*W2) @ silu(W1 @ X^T + b1) + scale*b2;
4. (host) scatter the projected rows into the zero-initialized output.

Device kernel notes (v2):
- fp16 operands: same 1 cycle/row PE rate as f32r but half the HBM
  traffic (~8MB/core), and ~4e-4 rel err vs the 2e-2 gate.
- All inputs prepacked on host to partition-major [128, ktiles*width]
  layouts so every DMA descriptor moves a multi-KB contiguous line
  (the v1 kernel's 624B lines ran DMA at ~184GB/s of the 360 peak).
- DMA issued in k-chunks (1,1,2,4,4,...) so the first matmul starts
  ~2.5us in and the PE then streams without starving (per 4-k-tile
  chunk: DMA ~3.1us vs PE ~3.7us).
- Tail: the last k-chunk of GEMM1 runs h-major so each silu(h) and its
  GEMM2 matmuls overlap the remaining GEMM1 work instead of forming a
  serial epilogue; output copies go on the vector engine to stay off
  the silu-busy scalar engine.
"""

import os
import sys

sys.path.insert(0, "/opt/trn_rl_repo")

import numpy as np

import concourse.bass as bass
import concourse.mybir as mybir
from concourse import bacc
from concourse.bass_utils import run_bass_kernel_spmd
from concourse.tile import TileContext

# Problem constants (hardcoded per contract)
B, L_TEXT, N_TOKEN = 8, 2048, 2048
C_TEXT, C_OUT = 3584, 384
C_HID = C_OUT * 2
CHAIN_TYPES = (1, 2)
CDR_TYPES = (2, 4, 6)
N_CORES = 8

KT = C_TEXT // 128   # 28 k-tiles (contraction of GEMM1)
HT = C_HID // 128    # 6 h-tiles
OT = C_OUT // 128    # 3 out-tiles

F32 = mybir.dt.float32
F16 = mybir.dt.float16
AF = mybir.ActivationFunctionType

_kernel_cache = {}

# test harness hooks: set _TRACE=True to profile; exec times land in
# _last_exec_ns (one entry per device launch).
_TRACE = False
_last_exec_ns = []
_last_results = []


def _chunk_sizes(kt_x: int) -> list:
    """k-tile DMA chunking: small leading chunks for a fast PE start,
    then 4-tile chunks that keep DMA slightly ahead of the PE."""
    sizes = [1, 1, 2]
    left = kt_x - sum(sizes)
    while left > 0:
        take = min(4, left)
        sizes.append(take)
        left -= take
    return sizes


def _build_mlp_kernel(cap: int, has_b1: bool, has_b2: bool):
    """Dense MLP on `cap` gathered rows per core, fp16 operands."""
    kt_x = KT + (1 if has_b1 else 0)   # augmented contraction tiles
    chunks = _chunk_sizes(kt_x)
    nch = len(chunks)

    stride = C_HID + cap   # per-k-tile block: [w1 768 | x cap]

    nc = bacc.Bacc("TRN2", target_bir_lowering=False, debug=False,
                   num_devices=N_CORES)
    # Combined partition-major layout: column k*stride+j of partition p
    # holds W1T[128k+p, j] for j<768, else X^T[128k+p, j-768].  One DMA
    # per k-chunk moves weights and activations together with fat (multi
    # KB) per-partition lines, and the single HWDGE ring self-paces.
    wxp = nc.declare_dram_parameter("wxp", [128, kt_x * stride], F16,
                                    isOutput=False)
    w2p = nc.declare_dram_parameter("w2p", [128, HT * C_OUT], F16,
                                    isOutput=False)
    if has_b2:
        b2 = nc.declare_dram_parameter("b2", [1, C_OUT], F32, isOutput=False)
    out = nc.declare_dram_parameter("out", [128, OT * cap], F16,
                                    isOutput=True)

    with TileContext(nc) as tc:
        with (
            tc.tile_pool(name="persist", bufs=1) as pp,
            tc.tile_pool(name="psum", bufs=1, space="PSUM") as pq,
        ):
            wx_sb = [pp.tile([128, n * stride], F16, name=f"wxc{i}",
                             tag=f"wxc{i}") for i, n in enumerate(chunks)]
            w2_sb = pp.tile([128, HT * C_OUT], F16, name="w2", tag="w2")
            h_sb = pp.tile([128, HT * cap], F16, name="h", tag="h")
            y_sb = pp.tile([128, OT * cap], F16, name="y", tag="y")
            if has_b2:
                b2_sb = pp.tile([1, C_OUT], F32, name="b2", tag="b2")
                ones_sb = pp.tile([1, cap], F32, name="ones", tag="ones")

            # DMA feed: one combined w1+x DMA per chunk, then w2, all on
            # the SP ring.  FIFO keeps w2 strictly behind every chunk (on
            # its own ring the scheduler hoists it to t=0 where it steals
            # packet slots from chunk 0; spliced mid-stream it delays the
            # later chunks) — and the PE is still ~4us from needing it
            # when the last chunk lands.
            cc = 0
            for i, n in enumerate(chunks):
                nc.sync.dma_start(out=wx_sb[i][:],
                                  in_=wxp[:, cc:cc + n * stride])
                cc += n * stride
            nc.sync.dma_start(out=w2_sb[:], in_=w2p[:])
            if has_b2:
                nc.scalar.dma_start(out=b2_sb[:], in_=b2[:])
                nc.gpsimd.memset(ones_sb[:], 1.0)

            # PE p-state warmup: tiny matmuls on a memset tile keep the
            # tensor engine busy through its DVFS ramp while the first
            # real k-chunk is still in flight.
            warm_n = int(os.environ.get("CDR_WARM_MM", "40"))
            if warm_n:
                warm_sb = pp.tile([128, 64], F16, name="warm", tag="warm")
                nc.vector.memset(warm_sb[:], 0.0)
                wps = pq.tile([1, 64], F32, name="wps", tag="wps")
                for _ in range(warm_n):
                    nc.tensor.matmul(wps[:], lhsT=warm_sb[:, 0:1],
                                     rhs=warm_sb[:], start=True, stop=True)

            ps1 = [pq.tile([128, cap], F32, name=f"ps1_{h}", tag=f"ps1_{h}")
                   for h in range(HT)]
            # ps2_c shares ps1_c's slot (released after silu(c) reads it),
            # keeping the pool within the 8 PSUM banks.
            ps2 = [pq.tile([128, cap], F32, name=f"ps2_{c}", tag=f"ps1_{c}")
                   for c in range(OT)]

            def g1mm(ci, j, h, start, stop):
                nc.tensor.matmul(
                    ps1[h][:],
                    lhsT=wx_sb[ci][:, j * stride + h * 128:
                                   j * stride + (h + 1) * 128],
                    rhs=wx_sb[ci][:, j * stride + C_HID:
                                  j * stride + C_HID + cap],
                    start=start, stop=stop,
                )

            def g2mm(h, lo=0, hi=None):
                hi = cap if hi is None else hi
                for c in range(OT):
                    nc.tensor.matmul(
                        ps2[c][:, lo:hi],
                        lhsT=w2_sb[:, h * C_OUT + c * 128:
                                   h * C_OUT + (c + 1) * 128],
                        rhs=h_sb[:, h * cap + lo:h * cap + hi],
                        start=(h == 0),
                        stop=(h == HT - 1) and not has_b2,
                    )

            # Phase A: k-outer / h-inner over all but the last chunk.
            kglob = 0
            for i, n in enumerate(chunks[:-1]):
                for j in range(n):
                    for h in range(HT):
                        g1mm(i, j, h, start=(kglob == 0), stop=False)
                    kglob += 1

            # Phase B: last chunk h-major; silu(h) and GEMM2(h) overlap
            # the remaining GEMM1 matmuls.  The final h runs in column
            # halves so its silu -> GEMM2 -> cast chain pipelines instead
            # of serializing at full width in the kernel tail.
            li, ln = nch - 1, chunks[-1]
            half = (cap // 2) & ~3
            for h in range(HT):
                for j in range(ln):
                    g1mm(li, j, h, start=False, stop=(j == ln - 1))
                if h < HT - 1:
                    nc.scalar.activation(h_sb[:, h * cap:(h + 1) * cap],
                                         ps1[h][:], AF.Silu)
                else:
                    nc.scalar.activation(h_sb[:, h * cap:h * cap + half],
                                         ps1[h][:, 0:half], AF.Silu)
                    nc.scalar.activation(h_sb[:, h * cap + half:(h + 1) * cap],
                                         ps1[h][:, half:cap], AF.Silu)
                if h >= 1:
                    g2mm(h - 1)
            g2mm(HT - 1, 0, half)
            g2mm(HT - 1, half, cap)

            out_rings = [nc.sync, nc.scalar, nc.sync]
            for c in range(OT):
                if has_b2:
                    nc.tensor.matmul(
                        ps2[c][:],
                        lhsT=b2_sb[:, c * 128:(c + 1) * 128],
                        rhs=ones_sb[:],
                        start=False, stop=True,
                    )
                nc.vector.tensor_copy(y_sb[:, c * cap:(c + 1) * cap],
                                      ps2[c][:])
                out_rings[c].dma_start(out=out[:, c * cap:(c + 1) * cap],
                                       in_=y_sb[:, c * cap:(c + 1) * cap])
    nc.compile()
    return nc


def _get_kernel(cap: int, has_b1: bool, has_b2: bool):
    key = (cap, has_b1, has_b2)
    if key not in _kernel_cache:
        _kernel_cache[key] = _build_mlp_kernel(cap, has_b1, has_b2)
    return _kernel_cache[key]


def _alignment_indices(text_mask, chain_type_ids, cdr_region_type_ids,
                       boltz_chain_type, boltz_region_type):
    """All (b, text_src, protein_dst) triples, reference semantics."""
    tm = text_mask.astype(bool)
    bs, srcs, dsts = [], [], []
    for b in range(B):
        for ct in CHAIN_TYPES:
            for rt in CDR_TYPES:
                tmask = (chain_type_ids[b] == ct) & (cdr_region_type_ids[b] == rt) & tm[b]
                pmask = (boltz_chain_type[b] == ct) & (boltz_region_type[b] == rt)
                ti = np.nonzero(tmask)[0]
                pi = np.nonzero(pmask)[0]
                k = min(ti.shape[0], pi.shape[0])
                if k:
                    bs.append(np.full(k, b, np.int64))
                    srcs.append(ti[:k])
                    dsts.append(pi[:k])
    if not bs:
        z = np.zeros(0, np.int64)
        return z, z, z
    return np.concatenate(bs), np.concatenate(srcs), np.concatenate(dsts)


def _pack_kmajor(arr_t, kt, width, dtype=np.float16):
    """[kt*128, width] -> [128, kt*width] partition-major packing."""
    a = np.asarray(arr_t, dtype)
    a = a.reshape(kt, 128, width).transpose(1, 0, 2).reshape(128, kt * width)
    return np.ascontiguousarray(a)


def kernel(text_conditioning, text_mask, chain_type_ids, cdr_region_type_ids,
           boltz_chain_type, boltz_region_type, W1, b1, W2, b2, scale):
    text_conditioning = np.asarray(text_conditioning, np.float32)
    W1 = np.asarray(W1, np.float32)
    b1v = np.asarray(b1, np.float32).reshape(-1)
    W2 = np.asarray(W2, np.float32)
    b2v = np.asarray(b2, np.float32).reshape(-1)
    scale_v = np.float32(np.asarray(scale).reshape(-1)[0])

    all_b, all_src, all_dst = _alignment_indices(
        np.asarray(text_mask), np.asarray(chain_type_ids),
        np.asarray(cdr_region_type_ids), np.asarray(boltz_chain_type),
        np.asarray(boltz_region_type))

    result = np.zeros((B, N_TOKEN, C_OUT), np.float32)
    nr = all_b.shape[0]
    if nr == 0:
        return result

    has_b1 = bool(b1v.any())
    b2s = b2v * scale_v
    has_b2 = bool(b2s.any())
    kt_x = KT + (1 if has_b1 else 0)

    # scale folds into the second layer
    w1T = np.ascontiguousarray(W1.T).astype(np.float16)   # [3584, 768]
    if has_b1:
        aug = np.zeros((128, C_HID), np.float16)
        aug[0] = b1v.astype(np.float16)
        w1T = np.concatenate([w1T, aug], axis=0)          # [3712, 768]
    w1_packed = _pack_kmajor(w1T, kt_x, C_HID)            # [128, kt_x*768]
    w2T = np.ascontiguousarray((W2 * scale_v).T).astype(np.float16)
    w2_packed = _pack_kmajor(w2T, HT, C_OUT)              # [128, 6*384]

    x_rows = text_conditioning[all_b, all_src, :].astype(np.float16)

    per_launch_cap = 512
    launch_rows = N_CORES * per_launch_cap
    y_rows = np.empty((nr, C_OUT), np.float32)

    for lo in range(0, nr, launch_rows):
        hi = min(nr, lo + launch_rows)
        per_core = -(-(hi - lo) // N_CORES)
        cap = min(per_launch_cap, max(64, -(-per_core // 4) * 4))
        nc = _get_kernel(cap, has_b1, has_b2)
        stride = C_HID + cap
        w1_blk = w1_packed.reshape(128, kt_x, C_HID)
        in_maps = []
        bounds = []
        for c in range(N_CORES):
            a = lo + c * cap
            z = min(hi, a + cap)
            a = min(a, z)
            bounds.append((a, z))
            wx = np.zeros((128, kt_x, stride), np.float16)
            wx[:, :, :C_HID] = w1_blk
            if z > a:
                wx[:, :KT, C_HID:C_HID + z - a] = (
                    x_rows[a:z].T.reshape(KT, 128, z - a).transpose(1, 0, 2))
                if has_b1:
                    wx[0, KT, C_HID:C_HID + z - a] = 1.0
            m = {"wxp": np.ascontiguousarray(wx.reshape(128, kt_x * stride)),
                 "w2p": w2_packed}
            if has_b2:
                m["b2"] = b2s.reshape(1, -1)
            in_maps.append(m)
        res = run_bass_kernel_spmd(nc, in_maps, list(range(N_CORES)),
                                   trace=_TRACE)
        if _TRACE:
            _last_exec_ns.append(res.exec_time_ns)
            _last_results.append(res)
        for c, (a, z) in enumerate(bounds):
            if z > a:
                o = res.results[c]["out"]                 # [128, OT*cap] f16
                y = o.reshape(128, OT, cap).transpose(1, 0, 2).reshape(
                    OT * 128, cap)
                y_rows[a:z] = y[:, :z - a].T.astype(np.float32)

    result[all_b, all_dst, :] = y_rows
    return result
